# revision 29
# baseline (speedup 1.0000x reference)
"""BiMamba block on 8 TRN2 NeuronCores.

Sharding: core = b*4 + q. Each core handles batch b and the q-th quarter
(384 channels) of d_inner for BOTH scan directions. The xp projection
partials are AllReduced (one collective for both directions) and the
out_proj partials ReduceScattered over each batch's 4-core group, so
every core finishes the block (residual + LN + FFN) on its own
256-token slice.

Implementation notes:
- LN gains/biases are folded into the adjacent matmul weights host-side.
- The backward direction is time-reversed once at xn (reversed-AP DVE
  copies); everything downstream runs in natural order and the final
  gating multiply writes its output reversed, so all scans run forward.
- The AllReduce payload is bf16 so B/C rows broadcast straight out of the
  collective's DRAM output via stride-0-source DMAs (no PE/PSUM hop).
- Per direction, the three 128-channel tiles are packed into one
  [128, 3L] "fat" tile so elementwise work runs as single wide ops;
  only the scans (whose recurrence must not cross tile boundaries) and
  per-channel-scalar ops run per L-slice.
- Scan work is split across engines: DVE runs the scans and most TT ops,
  Pool the C-multiplies, Act the exps.
"""
import sys
sys.path.insert(0, '/opt/trn_rl_repo')
import numpy as np
import ml_dtypes
import concourse.bass as bass
import concourse.tile as tile
from concourse import bacc, mybir
from concourse.bass_utils import run_bass_kernel_spmd

BF = mybir.dt.bfloat16
F32 = mybir.dt.float32
AL = mybir.AluOpType
ACTF = mybir.ActivationFunctionType
BF_NP = ml_dtypes.bfloat16

D_MODEL = 768
D_STATE = 16
D_INNER = 1536
DT_RANK = 48
B_SZ = 2
L = 1024
NQ = 4
DQ = D_INNER // NQ      # 384 channels per core per direction
NCORES = 8
TOK = L // NQ           # 256 tokens per core after ReduceScatter
GROUPS = [[0, 1, 2, 3], [4, 5, 6, 7]]
NH = L // 512           # matmul N halves

COLL_MODE = 'coll'      # 'coll' | 'nocoll' (nocoll: local copies, timing only)
RS_DTYPE = BF           # dtype for out_proj ReduceScatter payload
AR_DTYPE = BF           # dtype for xp AllReduce payload
N_REPS = 1              # emit the pipeline N times (timing amplification)

_CACHE = {}


def _bcast(src_row, parts=128):
    """Partition-broadcast AP: [1, N] -> [[0, parts], [1, N]] (DRAM src)."""
    return bass.AP(tensor=src_row.tensor, offset=src_row.offset,
                   ap=[[0, parts]] + [list(d) for d in src_row.ap[1:]])


def _rep3(t):
    """Repeat a [128, L] tile 3x along free dim via a stride-0 middle dim."""
    ap = t[:].ap
    return bass.AP(tensor=t[:].tensor, offset=t[:].offset,
                   ap=[list(ap[0]), [0, 3], list(ap[1])])


def _emit(nc, tc, psA, psB, dram, pp, rep, tens):
    (xh16, x_res, in_wT, convw, conv_bias, z_bias, xp_wT, dt_wT_f, dt_wT_b,
     dt_bias, d_skip, out_wT, w1T, b1_sh, w2T, b2_row, out) = tens
    L3 = 3 * L

    # ---------------- persistent small tensors ----------------
    convw_sb = pp.tile([128, 24], F32); nc.sync.dma_start(convw_sb[:], convw[:])
    conv_b_sb = pp.tile([128, 6], F32); nc.sync.dma_start(conv_b_sb[:], conv_bias[:])
    z_b_sb = pp.tile([128, 6], F32); nc.sync.dma_start(z_b_sb[:], z_bias[:])
    dt_b_sb = pp.tile([128, 6], F32); nc.sync.dma_start(dt_b_sb[:], dt_bias[:])
    d_skip_sb = pp.tile([128, 6], F32); nc.sync.dma_start(d_skip_sb[:], d_skip[:])
    b1_sb = pp.tile([128, 24], F32); nc.sync.dma_start(b1_sb[:], b1_sh[:])
    b2_32 = pp.tile([1, D_MODEL], F32); nc.sync.dma_start(b2_32[:], b2_row[:])
    b2_16 = pp.tile([1, D_MODEL], BF)
    nc.vector.tensor_copy(b2_16[:], b2_32[:])
    eps_sb = pp.tile([128, 1], F32); nc.vector.memset(eps_sb[:], 1e-5)
    ones_row = pp.tile([1, 128], BF); nc.vector.memset(ones_row[:], 1.0)
    ones_col = pp.tile([128, 1], BF); nc.vector.memset(ones_col[:], 1.0)

    xp_w_sb = pp.tile([128, 480], BF)
    nc.sync.dma_start(xp_w_sb[:], xp_wT[:])
    dtw_sb = []
    for t in (dt_wT_f, dt_wT_b):
        w = pp.tile([DT_RANK, DQ], BF)
        nc.sync.dma_start(w[:], t[:])
        dtw_sb.append(w)

    def replicate_ps(ps_tile, row_bf, width=L):
        for o in range(0, width, 512):
            w = min(512, width - o)
            nc.tensor.matmul(ps_tile[:, o:o + w], ones_row[:],
                             row_bf[0:1, o:o + w], start=True, stop=True)

    # persistent activations (fat per-direction tiles)
    z_fat = [pp.tile([128, L3], BF, tag=f"z{d}", name=f"z{d}") for d in range(2)]
    xcs_fat = [pp.tile([128, L3], BF, tag=f"xcs{d}", name=f"xcs{d}") for d in range(2)]
    delta_fat = [pp.tile([128, L3], BF, tag=f"dl{d}", name=f"dl{d}") for d in range(2)]
    dx_fat = [pp.tile([128, L3], BF, tag=f"dx{d}", name=f"dx{d}") for d in range(2)]
    acc_fat = [pp.tile([128, L3], BF, tag=f"acc{d}", name=f"acc{d}") for d in range(2)]
    yg_fat = dx_fat   # gate output reuses the dx tiles (dx dead by then)

    # collective DRAM buffers (single merged AllReduce measured faster than
    # one per direction)
    cc_in = dram.tile([160, L], AR_DTYPE, tag="cci", name="cci")
    cc_out = dram.tile([160, L], AR_DTYPE, tag="cco", name="cco")
    rs_in = dram.tile([L, D_MODEL], RS_DTYPE, tag="rsi", name="rsi")
    rs_out = dram.tile([TOK, D_MODEL], RS_DTYPE, tag="rso", name="rso")

    # ========== LN1 stats + xn + in_proj + conv + xp + delta ==========
    with tc.tile_pool(name=f"ln1_{rep}", bufs=1) as lp, \
         tc.tile_pool(name=f"work_{rep}", bufs=2) as wk:
        xh = [lp.tile([128, L], BF, tag=f"xh{k}", name=f"xh{k}")
              for k in range(6)]
        xnr = [lp.tile([128, L], BF, tag=f"xnr{k}", name=f"xnr{k}")
               for k in range(6)]
        psum_s = psA.tile([1, L], F32, tag="mm", name="lnred_s")
        psum_q = psA.tile([1, L], F32, tag="mm", name="lnred_q")
        for k in range(6):
            nc.sync.dma_start(xh[k][:], xh16[k * 128:(k + 1) * 128, :])
            sqb = wk.tile([128, L], BF, tag="sqb", name="sqb")
            nc.scalar.activation(sqb[:], xh[k][:], ACTF.Square)
            for nh in range(NH):
                nc.tensor.matmul(psum_s[:, nh * 512:(nh + 1) * 512],
                                 ones_col[:], xh[k][:, nh * 512:(nh + 1) * 512],
                                 start=(k == 0), stop=(k == 5))
                nc.tensor.matmul(psum_q[:, nh * 512:(nh + 1) * 512],
                                 ones_col[:], sqb[:, nh * 512:(nh + 1) * 512],
                                 start=(k == 0), stop=(k == 5))
        mean = wk.tile([1, L], F32, tag="vt", name="mean", bufs=3)
        nc.scalar.activation(mean[:], psum_s[:], ACTF.Copy, scale=1.0 / D_MODEL)
        mean16 = lp.tile([1, L], BF)
        nc.vector.tensor_copy(mean16[:], mean[:])
        e2 = wk.tile([1, L], F32, tag="vt", name="e2", bufs=3)
        nc.scalar.activation(e2[:], psum_q[:], ACTF.Copy, scale=1.0 / D_MODEL)
        var = wk.tile([1, L], F32, tag="vt", name="var", bufs=3)
        nc.vector.tensor_mul(var[:], mean[:], mean[:])
        nc.vector.tensor_sub(var[:], e2[:], var[:])
        sd = wk.tile([1, L], F32, tag="vt", name="sd", bufs=3)
        nc.scalar.activation(sd[:], var[:], ACTF.Sqrt, bias=eps_sb[0:1, :])
        rstd = wk.tile([1, L], F32, tag="vt", name="rstd", bufs=3)
        nc.vector.reciprocal(rstd[:], sd[:])
        rstd16 = lp.tile([1, L], BF)
        nc.vector.tensor_copy(rstd16[:], rstd[:])
        mrep_ps = psA.tile([128, L], F32, tag="mm", name="mrep_ps")
        replicate_ps(mrep_ps, mean16)
        mrep = lp.tile([128, L], BF)
        nc.scalar.activation(mrep[:], mrep_ps[:], ACTF.Copy)
        rrep_ps = psA.tile([128, L], F32, tag="mm", name="rrep_ps")
        replicate_ps(rrep_ps, rstd16)
        rrep = lp.tile([128, L], BF)
        nc.scalar.activation(rrep[:], rrep_ps[:], ACTF.Copy)
        xn = xh  # normalize in place
        for k in range(6):
            nc.vector.tensor_sub(xh[k][:], xh[k][:], mrep[:])
            nc.vector.tensor_mul(xh[k][:], xh[k][:], rrep[:])
            nc.vector.tensor_copy(xnr[k][:], xn[k][:, ::-1])

        # ---- in_proj ----
        xc_pad = [wk.tile([128, L + 4], BF, tag=f"xcp{i}", name=f"xcp{i}",
                          bufs=1)
                  for i in range(6)]
        for i in range(6):
            nc.vector.memset(xc_pad[i][:, 0:3], 0.0)
            nc.vector.memset(xc_pad[i][:, L + 3:L + 4], 0.0)
        # m order: 0-2 xc_f, 3-5 xc_b first; z tiles run during the AR wait
        with tc.tile_pool(name=f"iw_{rep}", bufs=1) as iwp:
            iwk = [iwp.tile([128, 4 * DQ], BF, tag=f"iwk{k}", name=f"iwk{k}")
                   for k in range(6)]
            for k in range(6):
                nc.sync.dma_start(iwk[k][:], in_wT[k * 128:(k + 1) * 128, :])

            def in_proj_tile(m):
                pm = psA.tile([128, L], F32, tag="mm", name="mm")
                grp, mt = divmod(m, 3)   # 0:xc_f 1:xc_b 2:z_f 3:z_b
                d = grp % 2
                rhs = xnr if d == 1 else xn
                for k in range(6):
                    for nh in range(NH):
                        nc.tensor.matmul(pm[:, nh * 512:(nh + 1) * 512],
                                         iwk[k][:, m * 128:(m + 1) * 128],
                                         rhs[k][:, nh * 512:(nh + 1) * 512],
                                         start=(k == 0), stop=(k == 5))
                i = d * 3 + mt
                if grp < 2:      # xc
                    nc.scalar.activation(xc_pad[i][:, 3:3 + L], pm[:], ACTF.Copy)
                else:            # z
                    nc.scalar.activation(z_fat[d][:, mt * L:(mt + 1) * L],
                                         pm[:], ACTF.Copy)

            for m in range(6):
                in_proj_tile(m)

            # ---- conv + silu; xp partials; one AllReduce for both dirs ----
            for d in range(2):
                for mt in range(3):
                    i = d * 3 + mt
                    tmp = wk.tile([128, L], F32, tag="cvt", name="cvt")
                    for j in range(4):
                        nc.vector.scalar_tensor_tensor(
                            tmp[:], xc_pad[i][:, j:j + L],
                            convw_sb[:, i * 4 + j:i * 4 + j + 1], tmp[:],
                            AL.mult, AL.bypass if j == 0 else AL.add)
                    nc.scalar.activation(xcs_fat[d][:, mt * L:(mt + 1) * L],
                                         tmp[:], ACTF.Silu,
                                         bias=conv_b_sb[:, i:i + 1])
                pxp = psB.tile([80, L], F32, tag="xp", name="xp")
                for kt in range(3):
                    for nh in range(NH):
                        nc.tensor.matmul(pxp[:, nh * 512:(nh + 1) * 512],
                                         xp_w_sb[:, d * 240 + kt * 80:
                                                 d * 240 + (kt + 1) * 80],
                                         xcs_fat[d][:, kt * L + nh * 512:
                                                    kt * L + (nh + 1) * 512],
                                         start=(kt == 0), stop=(kt == 2))
                sxp = wk.tile([80, L], AR_DTYPE, tag="sxp", name=f"sxp{d}")
                nc.scalar.activation(sxp[:], pxp[:], ACTF.Copy)
                nc.sync.dma_start(cc_in[d * 80:(d + 1) * 80, :], sxp[:])
            if COLL_MODE == 'nocoll':
                nc.sync.dma_start(cc_out[:], cc_in[:])
            else:
                nc.gpsimd.collective_compute("AllReduce", AL.add,
                                             replica_groups=GROUPS,
                                             ins=[cc_in.opt()],
                                             outs=[cc_out.opt()])

            # z-projection tiles fill the AllReduce wait
            for m in range(6, 12):
                in_proj_tile(m)

        # ---- post-AR: dt matmul + softplus + dx ----
        for d in range(2):
            dt16 = wk.tile([DT_RANK, L], BF, tag="dt16", name=f"dt16{d}")
            nc.sync.dma_start(dt16[:], cc_out[d * 80:d * 80 + DT_RANK, :])
            for mt in range(3):
                i = d * 3 + mt
                pdl = psA.tile([128, L], F32, tag="mm", name="mm")
                for nh in range(NH):
                    nc.tensor.matmul(pdl[:, nh * 512:(nh + 1) * 512],
                                     dtw_sb[d][:, mt * 128:(mt + 1) * 128],
                                     dt16[:, nh * 512:(nh + 1) * 512],
                                     start=True, stop=True)
                esp = wk.tile([128, L], F32, tag="esp", name="esp")
                nc.scalar.activation(esp[:], pdl[:], ACTF.Exp,
                                     bias=dt_b_sb[:, i:i + 1])
                nc.scalar.activation(delta_fat[d][:, mt * L:(mt + 1) * L],
                                     esp[:], ACTF.Ln, bias=1.0)
            nc.vector.tensor_mul(dx_fat[d][:], delta_fat[d][:], xcs_fat[d][:])

    # ====== late weights (DMAs overlap the scan phase, Act queue) ======
    with tc.tile_pool(name=f"wf_{rep}", bufs=1) as wf:
        outw_sb = [wf.tile([128, D_MODEL], BF, tag=f"outw{k}", name=f"outw{k}")
                   for k in range(6)]
        for k in range(6):
            nc.scalar.dma_start(outw_sb[k][:], out_wT[k * 128:(k + 1) * 128, :])
        w1_sb = [wf.tile([128, 4 * D_MODEL], BF, tag=f"w1s{k}", name=f"w1s{k}")
                 for k in range(6)]
        for k in range(6):
            nc.scalar.dma_start(w1_sb[k][:], w1T[k * 128:(k + 1) * 128, :])
        w2_sb = [wf.tile([128, 4 * D_MODEL], BF, tag=f"w2s{j}", name=f"w2s{j}")
                 for j in range(6)]
        for j in range(6):
            for kk in range(4):
                nc.scalar.dma_start(
                    w2_sb[j][:, kk * D_MODEL:(kk + 1) * D_MODEL],
                    w2T[(4 * j + kk) * 128:(4 * j + kk + 1) * 128, :])

        # ============== selective scan (all forward) ==============
        with tc.tile_pool(name=f"scan_{rep}", bufs=2) as sp, \
             tc.tile_pool(name=f"rep_{rep}", bufs=2) as rp:
            for d in range(2):
                # software-pipeline the acc += ch adds two states behind the
                # scans so DVE never waits on Pool's C-multiply
                pend = []
                for s in range(D_STATE):
                    brep = rp.tile([128, L], BF, tag="brep", name="brep")
                    nc.sync.dma_start(
                        brep[:],
                        _bcast(cc_out[d * 80 + DT_RANK + s:
                                      d * 80 + DT_RANK + s + 1, :]))
                    crep = rp.tile([128, L], BF, tag="crep", name="crep")
                    nc.sync.dma_start(
                        crep[:],
                        _bcast(cc_out[d * 80 + DT_RANK + D_STATE + s:
                                      d * 80 + DT_RANK + D_STATE + s + 1, :]))
                    dA = sp.tile([128, 3 * L], BF, tag="dA", name="dA")
                    nc.scalar.activation(dA[:], delta_fat[d][:], ACTF.Exp,
                                         scale=-(s + 1.0))
                    dBu = sp.tile([128, 3 * L], BF, tag="dBu", name="dBu")
                    nc.vector.tensor_tensor(dBu[:], dx_fat[d][:], _rep3(brep),
                                            AL.mult)
                    h = sp.tile([128, 3 * L], BF, tag="h", name="h")
                    for mt in range(3):
                        nc.vector.tensor_tensor_scan(
                            h[:, mt * L:(mt + 1) * L],
                            dA[:, mt * L:(mt + 1) * L],
                            dBu[:, mt * L:(mt + 1) * L],
                            0.0, AL.mult, AL.add)
                    if s == 0:
                        nc.gpsimd.tensor_tensor(acc_fat[d][:], h[:],
                                                _rep3(crep), AL.mult)
                    else:
                        ch = sp.tile([128, 3 * L], BF, tag="ch", name="ch")
                        nc.gpsimd.tensor_tensor(ch[:], h[:], _rep3(crep),
                                                AL.mult)
                        pend.append(ch)
                    if pend:
                        nc.vector.tensor_add(acc_fat[d][:], acc_fat[d][:],
                                             pend.pop(0)[:])

        # ---- gating + out_proj + ReduceScatter ----
        with tc.tile_pool(name=f"gate_{rep}", bufs=2) as gp:
            for d in range(2):
                for mt in range(3):
                    i = d * 3 + mt
                    sl_ = slice(mt * L, (mt + 1) * L)
                    tmp = gp.tile([128, L], BF, tag="gt", name="gt")
                    nc.vector.scalar_tensor_tensor(
                        tmp[:], xcs_fat[d][:, sl_], d_skip_sb[:, i:i + 1],
                        acc_fat[d][:, sl_], AL.mult, AL.add)
                    zs = gp.tile([128, L], BF, tag="zs", name="zs")
                    nc.scalar.activation(zs[:], z_fat[d][:, sl_], ACTF.Silu,
                                         bias=z_b_sb[:, i:i + 1])
                    if d == 0:
                        nc.vector.tensor_mul(yg_fat[d][:, sl_], tmp[:], zs[:])
                    else:
                        # backward dir: un-reverse while writing
                        nc.vector.tensor_mul(
                            yg_fat[d][:, (mt + 1) * L - 1:mt * L - 1 if mt else None:-1],
                            tmp[:], zs[:])

        with tc.tile_pool(name=f"opj_{rep}", bufs=2) as opj:
            for tt in range(8):
                po = psA.tile([128, D_MODEL], F32, tag="mm", name="po")
                for ki in range(6):
                    d, mt = divmod(ki, 3)
                    lhs = yg_fat[d][:, mt * L + tt * 128:mt * L + (tt + 1) * 128]
                    for o, w in ((0, 512), (512, 256)):
                        nc.tensor.matmul(po[:, o:o + w], lhs,
                                         outw_sb[ki][:, o:o + w],
                                         start=(ki == 0), stop=(ki == 5))
                so = opj.tile([128, D_MODEL], RS_DTYPE, tag="so", name="so")
                nc.scalar.activation(so[:], po[:], ACTF.Copy)
                nc.sync.dma_start(rs_in[tt * 128:(tt + 1) * 128, :], so[:])
        if COLL_MODE == 'nocoll':
            nc.sync.dma_start(rs_out[:], rs_in[0:TOK, :])
        else:
            nc.gpsimd.collective_compute("ReduceScatter", AL.add,
                                         replica_groups=GROUPS,
                                         ins=[rs_in.opt()], outs=[rs_out.opt()])

        # ======= residual + LN2 (token-major) + FFN =======
        with tc.tile_pool(name=f"ffn_{rep}", bufs=1) as fp:
            x2 = [fp.tile([128, D_MODEL], F32, tag=f"x2{t}", name=f"x2{t}")
                  for t in range(2)]
            for t in range(2):
                rsy = fp.tile([128, D_MODEL], RS_DTYPE, tag="rsy", name="rsy")
                nc.sync.dma_start(rsy[:], rs_out[t * 128:(t + 1) * 128, :])
                xr = fp.tile([128, D_MODEL], F32, tag="xr", name="xr")
                nc.sync.dma_start(xr[:], x_res[t * 128:(t + 1) * 128, :])
                nc.vector.tensor_add(x2[t][:], rsy[:], xr[:])
            xn2_bf = [fp.tile([128, D_MODEL], BF, tag=f"xn2{t}", name=f"xn2{t}")
                      for t in range(2)]
            for t in range(2):
                stats = fp.tile([128, 3, 6], F32, tag="bst", name="bst")
                for c in range(3):
                    nc.vector.bn_stats(stats[:, c, :],
                                       x2[t][:, c * 256:(c + 1) * 256])
                mv = fp.tile([128, 2], F32, tag="mv", name="mv")
                nc.vector.bn_aggr(mv[:], stats[:])
                sd2 = fp.tile([128, 1], F32, tag="sd2", name="sd2")
                nc.scalar.activation(sd2[:], mv[:, 1:2], ACTF.Sqrt,
                                     bias=eps_sb[:, 0:1])
                rstd2 = fp.tile([128, 1], F32, tag="rstd2", name="rstd2")
                nc.vector.reciprocal(rstd2[:], sd2[:])
                t1 = fp.tile([128, D_MODEL], F32, tag="ft1", name="ft1")
                nc.vector.tensor_scalar_sub(t1[:], x2[t][:], mv[:, 0:1])
                nc.vector.tensor_scalar_mul(xn2_bf[t][:], t1[:], rstd2[:])
            # transpose xn2 to feature-major via xbar DMA
            xn2_fm = [fp.tile([128, TOK], BF, tag=f"x2f{j}", name=f"x2f{j}")
                      for j in range(6)]
            for j in range(6):
                for t in range(2):
                    nc.sync.dma_start_transpose(
                        xn2_fm[j][:, t * 128:(t + 1) * 128],
                        xn2_bf[t][:, j * 128:(j + 1) * 128])
            # mm1 + gelu -> h_fm [3072, 256] bf16
            h_fm = [fp.tile([128, TOK], BF, tag=f"hf{m}", name=f"hf{m}")
                    for m in range(24)]
            for m in range(24):
                pf = psA.tile([128, TOK], F32, tag="mm", name="pf")
                for k in range(6):
                    nc.tensor.matmul(pf[:], w1_sb[k][:, m * 128:(m + 1) * 128],
                                     xn2_fm[k][:], start=(k == 0), stop=(k == 5))
                nc.scalar.activation(h_fm[m][:], pf[:], ACTF.Gelu,
                                     bias=b1_sb[:, m:m + 1])
            # mm2 (token-major out) with b2 as an augmented K row
            for t in range(2):
                po2 = psA.tile([128, D_MODEL], F32, tag="mm", name=f"po2{t}")
                for k in range(24):
                    j, kk = divmod(k, 4)
                    for o, w in ((0, 512), (512, 256)):
                        nc.tensor.matmul(
                            po2[:, o:o + w],
                            h_fm[k][:, t * 128:(t + 1) * 128],
                            w2_sb[j][:, kk * D_MODEL + o:kk * D_MODEL + o + w],
                            start=(k == 0), stop=False)
                for o, w in ((0, 512), (512, 256)):
                    nc.tensor.matmul(po2[:, o:o + w], ones_row[:],
                                     b2_16[0:1, o:o + w],
                                     start=False, stop=True)
                t4 = fp.tile([128, D_MODEL], F32, tag="t4", name="t4")
                nc.vector.tensor_add(t4[:], po2[:], x2[t][:])
                nc.sync.dma_start(out[t * 128:(t + 1) * 128, :], t4[:])


def build():
    nc = bacc.Bacc("TRN2", target_bir_lowering=False, debug=False,
                   num_devices=NCORES)

    def din(name, shape, dt=F32):
        return nc.dram_tensor(name, shape, dt, kind="ExternalInput")

    xh16 = din("xh16", [D_MODEL, L], BF)            # x[b].T  (bf16)
    x_res = din("x_res", [TOK, D_MODEL])            # token slice of x[b]
    in_wT = din("in_wT", [D_MODEL, 4 * DQ], BF)     # m: xc_f xc_b z_f z_b
    convw = din("convw", [128, 24])                 # (tile, tap)
    conv_bias = din("conv_bias", [128, 6])          # silu bias after conv
    z_bias = din("z_bias", [128, 6])                # silu bias for z
    xp_wT = din("xp_wT", [128, 480], BF)            # 2 dirs x 3 k-tiles
    dt_wT_f = din("dt_wT_f", [DT_RANK, DQ], BF)
    dt_wT_b = din("dt_wT_b", [DT_RANK, DQ], BF)
    dt_bias = din("dt_bias", [128, 6])
    d_skip = din("d_skip", [128, 6])
    out_wT = din("out_wT", [2 * DQ, D_MODEL], BF)   # rows: f then b, x0.5
    w1T = din("w1T", [D_MODEL, 4 * D_MODEL], BF)    # ln2-g folded
    b1_sh = din("b1_sh", [128, 24])                 # b1 + w1 @ ln2-b
    w2T = din("w2T", [4 * D_MODEL, D_MODEL], BF)
    b2_row = din("b2_row", [1, D_MODEL])
    out = nc.dram_tensor("out", [TOK, D_MODEL], F32, kind="ExternalOutput")
    tens = (xh16, x_res, in_wT, convw, conv_bias, z_bias, xp_wT, dt_wT_f,
            dt_wT_b, dt_bias, d_skip, out_wT, w1T, b1_sh, w2T, b2_row, out)

    with tile.TileContext(nc) as tc:
        with tc.tile_pool(name="psA", bufs=3, space="PSUM") as psA, \
             tc.tile_pool(name="psB", bufs=1, space="PSUM") as psB, \
             tc.tile_pool(name="dram", bufs=1, space="DRAM") as dram:
            for rep in range(N_REPS):
                with tc.tile_pool(name=f"persist_{rep}", bufs=1) as pp:
                    _emit(nc, tc, psA, psB, dram, pp, rep, tens)

    nc.compile()
    return nc


def _prep(inputs):
    f32 = np.float32
    x = np.asarray(inputs['x'], f32)
    ln_g = np.asarray(inputs['ln_g'], f32)
    ln_b = np.asarray(inputs['ln_b'], f32)
    g2 = np.asarray(inputs['ffn_ln_g'], f32)
    b2ln = np.asarray(inputs['ffn_ln_b'], f32)
    w1 = np.asarray(inputs['w1'], f32)
    b1 = np.asarray(inputs['b1'], f32)
    w2 = np.asarray(inputs['w2'], f32)
    b2 = np.asarray(inputs['b2'], f32)

    maps = []
    for core in range(NCORES):
        b, q = divmod(core, NQ)
        sl = slice(q * DQ, (q + 1) * DQ)

        def pp(v):  # (768,) -> (128, 6) per-partition columns
            return np.ascontiguousarray(v.reshape(6, 128).T.astype(f32))

        m = {}
        m['xh16'] = np.ascontiguousarray(x[b].T).astype(BF_NP)
        m['x_res'] = np.ascontiguousarray(x[b, q * TOK:(q + 1) * TOK])

        # in_proj weights with ln_g folded; column order xc_f xc_b z_f z_b
        iw_f = np.asarray(inputs['in_w_f'], f32)
        iw_b = np.asarray(inputs['in_w_b'], f32)
        zsl = slice(D_INNER + q * DQ, D_INNER + (q + 1) * DQ)
        xc_f_w = iw_f[sl] * ln_g[None, :]
        z_f_w = iw_f[zsl] * ln_g[None, :]
        xc_b_w = iw_b[sl] * ln_g[None, :]
        z_b_w = iw_b[zsl] * ln_g[None, :]
        m['in_wT'] = np.concatenate([xc_f_w, xc_b_w, z_f_w, z_b_w]).T.astype(BF_NP)
        # ln_b contribution (constant per channel)
        c0_xc_f = iw_f[sl] @ ln_b
        c0_z_f = iw_f[zsl] @ ln_b
        c0_xc_b = iw_b[sl] @ ln_b
        c0_z_b = iw_b[zsl] @ ln_b

        # conv: natural taps both dirs (bwd input is time-reversed)
        wf_ = np.asarray(inputs['conv_w_f'], f32)[sl, 0, :]
        wb_ = np.asarray(inputs['conv_w_b'], f32)[sl, 0, :]
        W = np.concatenate([wf_, wb_])
        cw = np.zeros((128, 24), f32)
        for i in range(6):
            cw[:, i * 4:(i + 1) * 4] = W[i * 128:(i + 1) * 128]
        m['convw'] = cw
        cb_f = np.asarray(inputs['conv_b_f'], f32)[sl] + c0_xc_f * wf_.sum(1)
        cb_b = np.asarray(inputs['conv_b_b'], f32)[sl] + c0_xc_b * wb_.sum(1)
        m['conv_bias'] = pp(np.concatenate([cb_f, cb_b]))
        m['z_bias'] = pp(np.concatenate([c0_z_f, c0_z_b]))

        def pack_xp(w):  # (80, 1536) -> [128, 240] (3 k-tiles of [128,80])
            wt = w[:, sl].T.astype(BF_NP)        # [384, 80]
            out_ = np.zeros((128, 240), BF_NP)
            for kt in range(3):
                out_[:, kt * 80:(kt + 1) * 80] = wt[kt * 128:(kt + 1) * 128]
            return out_
        m['xp_wT'] = np.concatenate(
            [pack_xp(np.asarray(inputs['xp_w_f'], f32)),
             pack_xp(np.asarray(inputs['xp_w_b'], f32))], axis=1)
        m['dt_wT_f'] = np.asarray(inputs['dt_w_f'], f32)[sl].T.astype(BF_NP)
        m['dt_wT_b'] = np.asarray(inputs['dt_w_b'], f32)[sl].T.astype(BF_NP)
        m['dt_bias'] = pp(np.concatenate([np.asarray(inputs['dt_b_f'], f32)[sl],
                                          np.asarray(inputs['dt_b_b'], f32)[sl]]))
        m['d_skip'] = pp(np.concatenate([np.asarray(inputs['D_f'], f32)[sl],
                                         np.asarray(inputs['D_b'], f32)[sl]]))
        ow = np.concatenate([np.asarray(inputs['out_w_f'], f32)[:, sl].T,
                             np.asarray(inputs['out_w_b'], f32)[:, sl].T]) * 0.5
        m['out_wT'] = ow.astype(BF_NP)

        # FFN with ln2 folds
        m['w1T'] = (w1 * g2[None, :]).T.astype(BF_NP)
        b1p = b1 + w1 @ b2ln
        m['b1_sh'] = np.ascontiguousarray(b1p.reshape(24, 128).T)
        m['w2T'] = w2.T.astype(BF_NP)
        m['b2_row'] = b2[None, :]
        maps.append({k: np.ascontiguousarray(v) for k, v in m.items()})
    return maps


def kernel(**inputs):
    if 'nc' not in _CACHE:
        _CACHE['nc'] = build()
    nc = _CACHE['nc']
    maps = _prep(inputs)
    res = run_bass_kernel_spmd(nc, maps, core_ids=list(range(NCORES)), trace=False)
    out = np.empty((B_SZ, L, D_MODEL), np.float32)
    for core in range(NCORES):
        b, q = divmod(core, NQ)
        out[b, q * TOK:(q + 1) * TOK] = res.results[core]['out']
    return out


# revision 31
# speedup vs baseline: 1.0229x; 1.0229x over previous
"""BiMamba block on 8 TRN2 NeuronCores.

Sharding: core = b*4 + q. Each core handles batch b and the q-th quarter
(384 channels) of d_inner for BOTH scan directions. The xp projection
partials are AllReduced (one collective for both directions) and the
out_proj partials ReduceScattered over each batch's 4-core group, so
every core finishes the block (residual + LN + FFN) on its own
256-token slice.

Implementation notes:
- LN gains/biases are folded into the adjacent matmul weights host-side.
- The backward direction is time-reversed once at xn (reversed-AP DVE
  copies); everything downstream runs in natural order and the final
  gating multiply writes its output reversed, so all scans run forward.
- The AllReduce payload is bf16 so B/C rows broadcast straight out of the
  collective's DRAM output via stride-0-source DMAs (no PE/PSUM hop).
- Per direction, the three 128-channel tiles are packed into one
  [128, 3L] "fat" tile so elementwise work runs as single wide ops;
  only the scans (whose recurrence must not cross tile boundaries) and
  per-channel-scalar ops run per L-slice.
- Scan work is split across engines: DVE runs the scans and most TT ops,
  Pool the C-multiplies, Act the exps.
"""
import sys
sys.path.insert(0, '/opt/trn_rl_repo')
import numpy as np
import ml_dtypes
import concourse.bass as bass
import concourse.tile as tile
from concourse import bacc, mybir
from concourse.bass_utils import run_bass_kernel_spmd

BF = mybir.dt.bfloat16
F32 = mybir.dt.float32
AL = mybir.AluOpType
ACTF = mybir.ActivationFunctionType
BF_NP = ml_dtypes.bfloat16

D_MODEL = 768
D_STATE = 16
D_INNER = 1536
DT_RANK = 48
B_SZ = 2
L = 1024
NQ = 4
DQ = D_INNER // NQ      # 384 channels per core per direction
NCORES = 8
TOK = L // NQ           # 256 tokens per core after ReduceScatter
GROUPS = [[0, 1, 2, 3], [4, 5, 6, 7]]
NH = L // 512           # matmul N halves

COLL_MODE = 'coll'      # 'coll' | 'nocoll' (nocoll: local copies, timing only)
RS_DTYPE = BF           # dtype for out_proj ReduceScatter payload
AR_DTYPE = BF           # dtype for xp AllReduce payload
N_REPS = 1              # emit the pipeline N times (timing amplification)

_CACHE = {}


def _bcast(src_row, parts=128):
    """Partition-broadcast AP: [1, N] -> [[0, parts], [1, N]] (DRAM src)."""
    return bass.AP(tensor=src_row.tensor, offset=src_row.offset,
                   ap=[[0, parts]] + [list(d) for d in src_row.ap[1:]])


def _rep3(t):
    """Repeat a [128, L] tile 3x along free dim via a stride-0 middle dim."""
    ap = t[:].ap
    return bass.AP(tensor=t[:].tensor, offset=t[:].offset,
                   ap=[list(ap[0]), [0, 3], list(ap[1])])


def _emit(nc, tc, psA, psB, dram, pp, rep, tens):
    (xh16, x_res, in_wT, convw, conv_bias, z_bias, xp_wT, dt_wT_f, dt_wT_b,
     dt_bias, d_skip, out_wT, w1T, b1_sh, w2T, b2_row, out) = tens
    L3 = 3 * L

    # ---------------- persistent small tensors ----------------
    convw_sb = pp.tile([128, 24], F32); nc.sync.dma_start(convw_sb[:], convw[:])
    conv_b_sb = pp.tile([128, 6], F32); nc.sync.dma_start(conv_b_sb[:], conv_bias[:])
    z_b_sb = pp.tile([128, 6], F32); nc.sync.dma_start(z_b_sb[:], z_bias[:])
    dt_b_sb = pp.tile([128, 6], F32); nc.sync.dma_start(dt_b_sb[:], dt_bias[:])
    d_skip_sb = pp.tile([128, 6], F32); nc.sync.dma_start(d_skip_sb[:], d_skip[:])
    b1_sb = pp.tile([128, 24], F32); nc.sync.dma_start(b1_sb[:], b1_sh[:])
    b2_32 = pp.tile([1, D_MODEL], F32); nc.sync.dma_start(b2_32[:], b2_row[:])
    b2_16 = pp.tile([1, D_MODEL], BF)
    nc.vector.tensor_copy(b2_16[:], b2_32[:])
    eps_sb = pp.tile([128, 1], F32); nc.vector.memset(eps_sb[:], 1e-5)
    ones_row = pp.tile([1, 128], BF); nc.vector.memset(ones_row[:], 1.0)
    ones_col = pp.tile([128, 1], BF); nc.vector.memset(ones_col[:], 1.0)

    xp_w_sb = pp.tile([128, 480], BF)
    nc.sync.dma_start(xp_w_sb[:], xp_wT[:])
    dtw_sb = []
    for t in (dt_wT_f, dt_wT_b):
        w = pp.tile([DT_RANK, DQ], BF)
        nc.sync.dma_start(w[:], t[:])
        dtw_sb.append(w)

    def replicate_ps(ps_tile, row_bf, width=L):
        for o in range(0, width, 512):
            w = min(512, width - o)
            nc.tensor.matmul(ps_tile[:, o:o + w], ones_row[:],
                             row_bf[0:1, o:o + w], start=True, stop=True)

    # persistent activations (fat per-direction tiles)
    z_fat = [pp.tile([128, L3], BF, tag=f"z{d}", name=f"z{d}") for d in range(2)]
    xcs_fat = [pp.tile([128, L3], BF, tag=f"xcs{d}", name=f"xcs{d}") for d in range(2)]
    delta_fat = [pp.tile([128, L3], BF, tag=f"dl{d}", name=f"dl{d}") for d in range(2)]
    dx_fat = [pp.tile([128, L3], BF, tag=f"dx{d}", name=f"dx{d}") for d in range(2)]
    acc_fat = [pp.tile([128, L3], BF, tag=f"acc{d}", name=f"acc{d}") for d in range(2)]
    yg_fat = dx_fat   # gate output reuses the dx tiles (dx dead by then)

    # collective DRAM buffers (single merged AllReduce measured faster than
    # one per direction)
    cc_in = dram.tile([160, L], AR_DTYPE, tag="cci", name="cci")
    cc_out = dram.tile([160, L], AR_DTYPE, tag="cco", name="cco")
    rs_in = dram.tile([L, D_MODEL], RS_DTYPE, tag="rsi", name="rsi")
    rs_out = dram.tile([TOK, D_MODEL], RS_DTYPE, tag="rso", name="rso")

    # ========== LN1 stats + xn + in_proj + conv + xp + delta ==========
    with tc.tile_pool(name=f"ln1_{rep}", bufs=1) as lp, \
         tc.tile_pool(name=f"work_{rep}", bufs=2) as wk:
        xh = [lp.tile([128, L], BF, tag=f"xh{k}", name=f"xh{k}")
              for k in range(6)]
        xnr = [lp.tile([128, L], BF, tag=f"xnr{k}", name=f"xnr{k}")
               for k in range(6)]
        psum_s = psA.tile([1, L], F32, tag="mm", name="lnred_s")
        psum_q = psA.tile([1, L], F32, tag="mm", name="lnred_q")
        for k in range(6):
            nc.sync.dma_start(xh[k][:], xh16[k * 128:(k + 1) * 128, :])
            sqb = wk.tile([128, L], BF, tag="sqb", name="sqb")
            nc.scalar.activation(sqb[:], xh[k][:], ACTF.Square)
            for nh in range(NH):
                nc.tensor.matmul(psum_s[:, nh * 512:(nh + 1) * 512],
                                 ones_col[:], xh[k][:, nh * 512:(nh + 1) * 512],
                                 start=(k == 0), stop=(k == 5))
                nc.tensor.matmul(psum_q[:, nh * 512:(nh + 1) * 512],
                                 ones_col[:], sqb[:, nh * 512:(nh + 1) * 512],
                                 start=(k == 0), stop=(k == 5))
        mean = wk.tile([1, L], F32, tag="vt", name="mean", bufs=3)
        nc.scalar.activation(mean[:], psum_s[:], ACTF.Copy, scale=1.0 / D_MODEL)
        mean16 = lp.tile([1, L], BF)
        nc.vector.tensor_copy(mean16[:], mean[:])
        e2 = wk.tile([1, L], F32, tag="vt", name="e2", bufs=3)
        nc.scalar.activation(e2[:], psum_q[:], ACTF.Copy, scale=1.0 / D_MODEL)
        var = wk.tile([1, L], F32, tag="vt", name="var", bufs=3)
        nc.vector.tensor_mul(var[:], mean[:], mean[:])
        nc.vector.tensor_sub(var[:], e2[:], var[:])
        sd = wk.tile([1, L], F32, tag="vt", name="sd", bufs=3)
        nc.scalar.activation(sd[:], var[:], ACTF.Sqrt, bias=eps_sb[0:1, :])
        rstd = wk.tile([1, L], F32, tag="vt", name="rstd", bufs=3)
        nc.vector.reciprocal(rstd[:], sd[:])
        rstd16 = lp.tile([1, L], BF)
        nc.vector.tensor_copy(rstd16[:], rstd[:])
        mrep_ps = psA.tile([128, L], F32, tag="mm", name="mrep_ps")
        replicate_ps(mrep_ps, mean16)
        mrep = lp.tile([128, L], BF)
        nc.scalar.activation(mrep[:], mrep_ps[:], ACTF.Copy)
        rrep_ps = psA.tile([128, L], F32, tag="mm", name="rrep_ps")
        replicate_ps(rrep_ps, rstd16)
        rrep = lp.tile([128, L], BF)
        nc.scalar.activation(rrep[:], rrep_ps[:], ACTF.Copy)
        xn = xh  # normalize in place
        for k in range(6):
            nc.vector.tensor_sub(xh[k][:], xh[k][:], mrep[:])
            nc.vector.tensor_mul(xh[k][:], xh[k][:], rrep[:])
            nc.vector.tensor_copy(xnr[k][:], xn[k][:, ::-1])

        # ---- in_proj ----
        xc_pad = [wk.tile([128, L + 4], BF, tag=f"xcp{i}", name=f"xcp{i}",
                          bufs=1)
                  for i in range(6)]
        for i in range(6):
            nc.vector.memset(xc_pad[i][:, 0:3], 0.0)
            nc.vector.memset(xc_pad[i][:, L + 3:L + 4], 0.0)
        # m order: 0-2 xc_f, 3-5 xc_b first; z tiles run during the AR wait
        with tc.tile_pool(name=f"iw_{rep}", bufs=1) as iwp:
            iwk = [iwp.tile([128, 4 * DQ], BF, tag=f"iwk{k}", name=f"iwk{k}")
                   for k in range(6)]
            for k in range(6):
                nc.sync.dma_start(iwk[k][:], in_wT[k * 128:(k + 1) * 128, :])

            def in_proj_tile(m):
                pm = psA.tile([128, L], F32, tag="mm", name="mm")
                grp, mt = divmod(m, 3)   # 0:xc_f 1:xc_b 2:z_f 3:z_b
                d = grp % 2
                rhs = xnr if d == 1 else xn
                for k in range(6):
                    for nh in range(NH):
                        nc.tensor.matmul(pm[:, nh * 512:(nh + 1) * 512],
                                         iwk[k][:, m * 128:(m + 1) * 128],
                                         rhs[k][:, nh * 512:(nh + 1) * 512],
                                         start=(k == 0), stop=(k == 5))
                i = d * 3 + mt
                if grp < 2:      # xc
                    nc.scalar.activation(xc_pad[i][:, 3:3 + L], pm[:], ACTF.Copy)
                else:            # z
                    nc.scalar.activation(z_fat[d][:, mt * L:(mt + 1) * L],
                                         pm[:], ACTF.Copy)

            for m in range(12):
                in_proj_tile(m)

            # ---- conv + silu; xp partials; one AllReduce for both dirs ----
            for d in range(2):
                for mt in range(3):
                    i = d * 3 + mt
                    tmp = wk.tile([128, L], F32, tag="cvt", name="cvt")
                    for j in range(4):
                        nc.vector.scalar_tensor_tensor(
                            tmp[:], xc_pad[i][:, j:j + L],
                            convw_sb[:, i * 4 + j:i * 4 + j + 1], tmp[:],
                            AL.mult, AL.bypass if j == 0 else AL.add)
                    nc.scalar.activation(xcs_fat[d][:, mt * L:(mt + 1) * L],
                                         tmp[:], ACTF.Silu,
                                         bias=conv_b_sb[:, i:i + 1])
                pxp = psB.tile([80, L], F32, tag="xp", name="xp")
                for kt in range(3):
                    for nh in range(NH):
                        nc.tensor.matmul(pxp[:, nh * 512:(nh + 1) * 512],
                                         xp_w_sb[:, d * 240 + kt * 80:
                                                 d * 240 + (kt + 1) * 80],
                                         xcs_fat[d][:, kt * L + nh * 512:
                                                    kt * L + (nh + 1) * 512],
                                         start=(kt == 0), stop=(kt == 2))
                sxp = wk.tile([80, L], AR_DTYPE, tag="sxp", name=f"sxp{d}")
                nc.scalar.activation(sxp[:], pxp[:], ACTF.Copy)
                nc.sync.dma_start(cc_in[d * 80:(d + 1) * 80, :], sxp[:])
            if COLL_MODE == 'nocoll':
                nc.sync.dma_start(cc_out[:], cc_in[:])
            else:
                nc.gpsimd.collective_compute("AllReduce", AL.add,
                                             replica_groups=GROUPS,
                                             ins=[cc_in.opt()],
                                             outs=[cc_out.opt()])

        # ---- post-AR: dt matmul + softplus + dx ----
        for d in range(2):
            dt16 = wk.tile([DT_RANK, L], BF, tag="dt16", name=f"dt16{d}")
            nc.sync.dma_start(dt16[:], cc_out[d * 80:d * 80 + DT_RANK, :])
            for mt in range(3):
                i = d * 3 + mt
                pdl = psA.tile([128, L], F32, tag="mm", name="mm")
                for nh in range(NH):
                    nc.tensor.matmul(pdl[:, nh * 512:(nh + 1) * 512],
                                     dtw_sb[d][:, mt * 128:(mt + 1) * 128],
                                     dt16[:, nh * 512:(nh + 1) * 512],
                                     start=True, stop=True)
                esp = wk.tile([128, L], F32, tag="esp", name="esp")
                nc.scalar.activation(esp[:], pdl[:], ACTF.Exp,
                                     bias=dt_b_sb[:, i:i + 1])
                nc.scalar.activation(delta_fat[d][:, mt * L:(mt + 1) * L],
                                     esp[:], ACTF.Ln, bias=1.0)
            nc.vector.tensor_mul(dx_fat[d][:], delta_fat[d][:], xcs_fat[d][:])

    # ====== late weights (DMAs overlap the scan phase, Act queue) ======
    with tc.tile_pool(name=f"wf_{rep}", bufs=1) as wf:
        outw_sb = [wf.tile([128, D_MODEL], BF, tag=f"outw{k}", name=f"outw{k}")
                   for k in range(6)]
        for k in range(6):
            nc.scalar.dma_start(outw_sb[k][:], out_wT[k * 128:(k + 1) * 128, :])
        w1_sb = [wf.tile([128, 4 * D_MODEL], BF, tag=f"w1s{k}", name=f"w1s{k}")
                 for k in range(6)]
        for k in range(6):
            nc.scalar.dma_start(w1_sb[k][:], w1T[k * 128:(k + 1) * 128, :])
        w2_sb = [wf.tile([128, 4 * D_MODEL], BF, tag=f"w2s{j}", name=f"w2s{j}")
                 for j in range(6)]
        for j in range(6):
            for kk in range(4):
                nc.scalar.dma_start(
                    w2_sb[j][:, kk * D_MODEL:(kk + 1) * D_MODEL],
                    w2T[(4 * j + kk) * 128:(4 * j + kk + 1) * 128, :])

        # ============== selective scan (all forward) ==============
        with tc.tile_pool(name=f"scan_{rep}", bufs=2) as sp, \
             tc.tile_pool(name=f"rep_{rep}", bufs=2) as rp:
            for d in range(2):
                # software-pipeline the acc += ch adds two states behind the
                # scans so DVE never waits on Pool's C-multiply
                pend = []
                for s in range(D_STATE):
                    brep = rp.tile([128, L], BF, tag="brep", name="brep")
                    nc.sync.dma_start(
                        brep[:],
                        _bcast(cc_out[d * 80 + DT_RANK + s:
                                      d * 80 + DT_RANK + s + 1, :]))
                    crep = rp.tile([128, L], BF, tag="crep", name="crep")
                    nc.sync.dma_start(
                        crep[:],
                        _bcast(cc_out[d * 80 + DT_RANK + D_STATE + s:
                                      d * 80 + DT_RANK + D_STATE + s + 1, :]))
                    dA = sp.tile([128, 3 * L], BF, tag="dA", name="dA")
                    nc.scalar.activation(dA[:], delta_fat[d][:], ACTF.Exp,
                                         scale=-(s + 1.0))
                    dBu = sp.tile([128, 3 * L], BF, tag="dBu", name="dBu")
                    nc.vector.tensor_tensor(dBu[:], dx_fat[d][:], _rep3(brep),
                                            AL.mult)
                    h = sp.tile([128, 3 * L], BF, tag="h", name="h")
                    for mt in range(3):
                        nc.vector.tensor_tensor_scan(
                            h[:, mt * L:(mt + 1) * L],
                            dA[:, mt * L:(mt + 1) * L],
                            dBu[:, mt * L:(mt + 1) * L],
                            0.0, AL.mult, AL.add)
                    if s == 0:
                        nc.gpsimd.tensor_tensor(acc_fat[d][:], h[:],
                                                _rep3(crep), AL.mult)
                    else:
                        ch = sp.tile([128, 3 * L], BF, tag="ch", name="ch")
                        nc.gpsimd.tensor_tensor(ch[:], h[:], _rep3(crep),
                                                AL.mult)
                        pend.append(ch)
                    if pend:
                        nc.vector.tensor_add(acc_fat[d][:], acc_fat[d][:],
                                             pend.pop(0)[:])

        # ---- gating + out_proj + ReduceScatter ----
        with tc.tile_pool(name=f"gate_{rep}", bufs=2) as gp:
            for d in range(2):
                for mt in range(3):
                    i = d * 3 + mt
                    sl_ = slice(mt * L, (mt + 1) * L)
                    tmp = gp.tile([128, L], BF, tag="gt", name="gt")
                    nc.vector.scalar_tensor_tensor(
                        tmp[:], xcs_fat[d][:, sl_], d_skip_sb[:, i:i + 1],
                        acc_fat[d][:, sl_], AL.mult, AL.add)
                    zs = gp.tile([128, L], BF, tag="zs", name="zs")
                    nc.scalar.activation(zs[:], z_fat[d][:, sl_], ACTF.Silu,
                                         bias=z_b_sb[:, i:i + 1])
                    if d == 0:
                        nc.vector.tensor_mul(yg_fat[d][:, sl_], tmp[:], zs[:])
                    else:
                        # backward dir: un-reverse while writing
                        nc.vector.tensor_mul(
                            yg_fat[d][:, (mt + 1) * L - 1:mt * L - 1 if mt else None:-1],
                            tmp[:], zs[:])

        with tc.tile_pool(name=f"opj_{rep}", bufs=2) as opj:
            for tt in range(8):
                po = psA.tile([128, D_MODEL], F32, tag="mm", name="po")
                for ki in range(6):
                    d, mt = divmod(ki, 3)
                    lhs = yg_fat[d][:, mt * L + tt * 128:mt * L + (tt + 1) * 128]
                    for o, w in ((0, 512), (512, 256)):
                        nc.tensor.matmul(po[:, o:o + w], lhs,
                                         outw_sb[ki][:, o:o + w],
                                         start=(ki == 0), stop=(ki == 5))
                so = opj.tile([128, D_MODEL], RS_DTYPE, tag="so", name="so")
                nc.scalar.activation(so[:], po[:], ACTF.Copy)
                nc.sync.dma_start(rs_in[tt * 128:(tt + 1) * 128, :], so[:])
        if COLL_MODE == 'nocoll':
            nc.sync.dma_start(rs_out[:], rs_in[0:TOK, :])
        else:
            nc.gpsimd.collective_compute("ReduceScatter", AL.add,
                                         replica_groups=GROUPS,
                                         ins=[rs_in.opt()], outs=[rs_out.opt()])

        # ======= residual + LN2 (token-major) + FFN =======
        with tc.tile_pool(name=f"ffn_{rep}", bufs=1) as fp:
            x2 = [fp.tile([128, D_MODEL], F32, tag=f"x2{t}", name=f"x2{t}")
                  for t in range(2)]
            for t in range(2):
                rsy = fp.tile([128, D_MODEL], RS_DTYPE, tag="rsy", name="rsy")
                nc.sync.dma_start(rsy[:], rs_out[t * 128:(t + 1) * 128, :])
                xr = fp.tile([128, D_MODEL], F32, tag="xr", name="xr")
                nc.sync.dma_start(xr[:], x_res[t * 128:(t + 1) * 128, :])
                nc.vector.tensor_add(x2[t][:], rsy[:], xr[:])
            xn2_bf = [fp.tile([128, D_MODEL], BF, tag=f"xn2{t}", name=f"xn2{t}")
                      for t in range(2)]
            for t in range(2):
                stats = fp.tile([128, 3, 6], F32, tag="bst", name="bst")
                for c in range(3):
                    nc.vector.bn_stats(stats[:, c, :],
                                       x2[t][:, c * 256:(c + 1) * 256])
                mv = fp.tile([128, 2], F32, tag="mv", name="mv")
                nc.vector.bn_aggr(mv[:], stats[:])
                sd2 = fp.tile([128, 1], F32, tag="sd2", name="sd2")
                nc.scalar.activation(sd2[:], mv[:, 1:2], ACTF.Sqrt,
                                     bias=eps_sb[:, 0:1])
                rstd2 = fp.tile([128, 1], F32, tag="rstd2", name="rstd2")
                nc.vector.reciprocal(rstd2[:], sd2[:])
                t1 = fp.tile([128, D_MODEL], F32, tag="ft1", name="ft1")
                nc.vector.tensor_scalar_sub(t1[:], x2[t][:], mv[:, 0:1])
                nc.vector.tensor_scalar_mul(xn2_bf[t][:], t1[:], rstd2[:])
            # transpose xn2 to feature-major via xbar DMA
            xn2_fm = [fp.tile([128, TOK], BF, tag=f"x2f{j}", name=f"x2f{j}")
                      for j in range(6)]
            for j in range(6):
                for t in range(2):
                    nc.sync.dma_start_transpose(
                        xn2_fm[j][:, t * 128:(t + 1) * 128],
                        xn2_bf[t][:, j * 128:(j + 1) * 128])
            # mm1 + gelu -> h_fm [3072, 256] bf16
            h_fm = [fp.tile([128, TOK], BF, tag=f"hf{m}", name=f"hf{m}")
                    for m in range(24)]
            for m in range(24):
                pf = psA.tile([128, TOK], F32, tag="mm", name="pf")
                for k in range(6):
                    nc.tensor.matmul(pf[:], w1_sb[k][:, m * 128:(m + 1) * 128],
                                     xn2_fm[k][:], start=(k == 0), stop=(k == 5))
                nc.scalar.activation(h_fm[m][:], pf[:], ACTF.Gelu,
                                     bias=b1_sb[:, m:m + 1])
            # mm2 (token-major out) with b2 as an augmented K row
            for t in range(2):
                po2 = psA.tile([128, D_MODEL], F32, tag="mm", name=f"po2{t}")
                for k in range(24):
                    j, kk = divmod(k, 4)
                    for o, w in ((0, 512), (512, 256)):
                        nc.tensor.matmul(
                            po2[:, o:o + w],
                            h_fm[k][:, t * 128:(t + 1) * 128],
                            w2_sb[j][:, kk * D_MODEL + o:kk * D_MODEL + o + w],
                            start=(k == 0), stop=False)
                for o, w in ((0, 512), (512, 256)):
                    nc.tensor.matmul(po2[:, o:o + w], ones_row[:],
                                     b2_16[0:1, o:o + w],
                                     start=False, stop=True)
                t4 = fp.tile([128, D_MODEL], F32, tag="t4", name="t4")
                nc.vector.tensor_add(t4[:], po2[:], x2[t][:])
                nc.sync.dma_start(out[t * 128:(t + 1) * 128, :], t4[:])


def build():
    nc = bacc.Bacc("TRN2", target_bir_lowering=False, debug=False,
                   num_devices=NCORES)

    def din(name, shape, dt=F32):
        return nc.dram_tensor(name, shape, dt, kind="ExternalInput")

    xh16 = din("xh16", [D_MODEL, L], BF)            # x[b].T  (bf16)
    x_res = din("x_res", [TOK, D_MODEL])            # token slice of x[b]
    in_wT = din("in_wT", [D_MODEL, 4 * DQ], BF)     # m: xc_f xc_b z_f z_b
    convw = din("convw", [128, 24])                 # (tile, tap)
    conv_bias = din("conv_bias", [128, 6])          # silu bias after conv
    z_bias = din("z_bias", [128, 6])                # silu bias for z
    xp_wT = din("xp_wT", [128, 480], BF)            # 2 dirs x 3 k-tiles
    dt_wT_f = din("dt_wT_f", [DT_RANK, DQ], BF)
    dt_wT_b = din("dt_wT_b", [DT_RANK, DQ], BF)
    dt_bias = din("dt_bias", [128, 6])
    d_skip = din("d_skip", [128, 6])
    out_wT = din("out_wT", [2 * DQ, D_MODEL], BF)   # rows: f then b, x0.5
    w1T = din("w1T", [D_MODEL, 4 * D_MODEL], BF)    # ln2-g folded
    b1_sh = din("b1_sh", [128, 24])                 # b1 + w1 @ ln2-b
    w2T = din("w2T", [4 * D_MODEL, D_MODEL], BF)
    b2_row = din("b2_row", [1, D_MODEL])
    out = nc.dram_tensor("out", [TOK, D_MODEL], F32, kind="ExternalOutput")
    tens = (xh16, x_res, in_wT, convw, conv_bias, z_bias, xp_wT, dt_wT_f,
            dt_wT_b, dt_bias, d_skip, out_wT, w1T, b1_sh, w2T, b2_row, out)

    with tile.TileContext(nc) as tc:
        with tc.tile_pool(name="psA", bufs=3, space="PSUM") as psA, \
             tc.tile_pool(name="psB", bufs=1, space="PSUM") as psB, \
             tc.tile_pool(name="dram", bufs=1, space="DRAM") as dram:
            for rep in range(N_REPS):
                with tc.tile_pool(name=f"persist_{rep}", bufs=1) as pp:
                    _emit(nc, tc, psA, psB, dram, pp, rep, tens)

    nc.compile()
    return nc


def _prep(inputs):
    f32 = np.float32
    x = np.asarray(inputs['x'], f32)
    ln_g = np.asarray(inputs['ln_g'], f32)
    ln_b = np.asarray(inputs['ln_b'], f32)
    g2 = np.asarray(inputs['ffn_ln_g'], f32)
    b2ln = np.asarray(inputs['ffn_ln_b'], f32)
    w1 = np.asarray(inputs['w1'], f32)
    b1 = np.asarray(inputs['b1'], f32)
    w2 = np.asarray(inputs['w2'], f32)
    b2 = np.asarray(inputs['b2'], f32)

    maps = []
    for core in range(NCORES):
        b, q = divmod(core, NQ)
        sl = slice(q * DQ, (q + 1) * DQ)

        def pp(v):  # (768,) -> (128, 6) per-partition columns
            return np.ascontiguousarray(v.reshape(6, 128).T.astype(f32))

        m = {}
        m['xh16'] = np.ascontiguousarray(x[b].T).astype(BF_NP)
        m['x_res'] = np.ascontiguousarray(x[b, q * TOK:(q + 1) * TOK])

        # in_proj weights with ln_g folded; column order xc_f xc_b z_f z_b
        iw_f = np.asarray(inputs['in_w_f'], f32)
        iw_b = np.asarray(inputs['in_w_b'], f32)
        zsl = slice(D_INNER + q * DQ, D_INNER + (q + 1) * DQ)
        xc_f_w = iw_f[sl] * ln_g[None, :]
        z_f_w = iw_f[zsl] * ln_g[None, :]
        xc_b_w = iw_b[sl] * ln_g[None, :]
        z_b_w = iw_b[zsl] * ln_g[None, :]
        m['in_wT'] = np.concatenate([xc_f_w, xc_b_w, z_f_w, z_b_w]).T.astype(BF_NP)
        # ln_b contribution (constant per channel)
        c0_xc_f = iw_f[sl] @ ln_b
        c0_z_f = iw_f[zsl] @ ln_b
        c0_xc_b = iw_b[sl] @ ln_b
        c0_z_b = iw_b[zsl] @ ln_b

        # conv: natural taps both dirs (bwd input is time-reversed)
        wf_ = np.asarray(inputs['conv_w_f'], f32)[sl, 0, :]
        wb_ = np.asarray(inputs['conv_w_b'], f32)[sl, 0, :]
        W = np.concatenate([wf_, wb_])
        cw = np.zeros((128, 24), f32)
        for i in range(6):
            cw[:, i * 4:(i + 1) * 4] = W[i * 128:(i + 1) * 128]
        m['convw'] = cw
        cb_f = np.asarray(inputs['conv_b_f'], f32)[sl] + c0_xc_f * wf_.sum(1)
        cb_b = np.asarray(inputs['conv_b_b'], f32)[sl] + c0_xc_b * wb_.sum(1)
        m['conv_bias'] = pp(np.concatenate([cb_f, cb_b]))
        m['z_bias'] = pp(np.concatenate([c0_z_f, c0_z_b]))

        def pack_xp(w):  # (80, 1536) -> [128, 240] (3 k-tiles of [128,80])
            wt = w[:, sl].T.astype(BF_NP)        # [384, 80]
            out_ = np.zeros((128, 240), BF_NP)
            for kt in range(3):
                out_[:, kt * 80:(kt + 1) * 80] = wt[kt * 128:(kt + 1) * 128]
            return out_
        m['xp_wT'] = np.concatenate(
            [pack_xp(np.asarray(inputs['xp_w_f'], f32)),
             pack_xp(np.asarray(inputs['xp_w_b'], f32))], axis=1)
        m['dt_wT_f'] = np.asarray(inputs['dt_w_f'], f32)[sl].T.astype(BF_NP)
        m['dt_wT_b'] = np.asarray(inputs['dt_w_b'], f32)[sl].T.astype(BF_NP)
        m['dt_bias'] = pp(np.concatenate([np.asarray(inputs['dt_b_f'], f32)[sl],
                                          np.asarray(inputs['dt_b_b'], f32)[sl]]))
        m['d_skip'] = pp(np.concatenate([np.asarray(inputs['D_f'], f32)[sl],
                                         np.asarray(inputs['D_b'], f32)[sl]]))
        ow = np.concatenate([np.asarray(inputs['out_w_f'], f32)[:, sl].T,
                             np.asarray(inputs['out_w_b'], f32)[:, sl].T]) * 0.5
        m['out_wT'] = ow.astype(BF_NP)

        # FFN with ln2 folds
        m['w1T'] = (w1 * g2[None, :]).T.astype(BF_NP)
        b1p = b1 + w1 @ b2ln
        m['b1_sh'] = np.ascontiguousarray(b1p.reshape(24, 128).T)
        m['w2T'] = w2.T.astype(BF_NP)
        m['b2_row'] = b2[None, :]
        maps.append({k: np.ascontiguousarray(v) for k, v in m.items()})
    return maps


def kernel(**inputs):
    if 'nc' not in _CACHE:
        _CACHE['nc'] = build()
    nc = _CACHE['nc']
    maps = _prep(inputs)
    res = run_bass_kernel_spmd(nc, maps, core_ids=list(range(NCORES)), trace=False)
    out = np.empty((B_SZ, L, D_MODEL), np.float32)
    for core in range(NCORES):
        b, q = divmod(core, NQ)
        out[b, q * TOK:(q + 1) * TOK] = res.results[core]['out']
    return out


# revision 35
# speedup vs baseline: 6.9979x; 6.8412x over previous
"""BiMamba block on 8 TRN2 NeuronCores.

Sharding: core = b*4 + q. Each core handles batch b and the q-th quarter
(384 channels) of d_inner for BOTH scan directions. The xp projection
partials are AllReduced (one collective for both directions) and the
out_proj partials ReduceScattered over each batch's 4-core group, so
every core finishes the block (residual + LN + FFN) on its own
256-token slice.

Implementation notes:
- LN gains/biases are folded into the adjacent matmul weights host-side.
- The backward direction is time-reversed once at xn (reversed-AP DVE
  copies); everything downstream runs in natural order and the final
  gating multiply writes its output reversed, so all scans run forward.
- The AllReduce payload is bf16 so B/C rows broadcast straight out of the
  collective's DRAM output via stride-0-source DMAs (no PE/PSUM hop).
- Per direction, the three 128-channel tiles are packed into one
  [128, 3L] "fat" tile so elementwise work runs as single wide ops;
  only the scans (whose recurrence must not cross tile boundaries) and
  per-channel-scalar ops run per L-slice.
- Scan work is split across engines: DVE runs the scans and most TT ops,
  Pool the C-multiplies, Act the exps.
"""
import sys
sys.path.insert(0, '/opt/trn_rl_repo')
import numpy as np
import ml_dtypes
import concourse.bass as bass
import concourse.tile as tile
from concourse import bacc, mybir
from concourse.bass_utils import run_bass_kernel_spmd

BF = mybir.dt.bfloat16
F32 = mybir.dt.float32
AL = mybir.AluOpType
ACTF = mybir.ActivationFunctionType
BF_NP = ml_dtypes.bfloat16

D_MODEL = 768
D_STATE = 16
D_INNER = 1536
DT_RANK = 48
B_SZ = 2
L = 1024
NQ = 4
DQ = D_INNER // NQ      # 384 channels per core per direction
NCORES = 8
TOK = L // NQ           # 256 tokens per core after ReduceScatter
GROUPS = [[0, 1, 2, 3], [4, 5, 6, 7]]
NH = L // 512           # matmul N halves

COLL_MODE = 'coll'      # 'coll' | 'nocoll' (nocoll: local copies, timing only)
RS_DTYPE = BF           # dtype for out_proj ReduceScatter payload
AR_DTYPE = BF           # dtype for xp AllReduce payload
N_REPS = 1              # emit the pipeline N times (timing amplification)

_CACHE = {}


def _bcast(src_row, parts=128):
    """Partition-broadcast AP: [1, N] -> [[0, parts], [1, N]] (DRAM src)."""
    return bass.AP(tensor=src_row.tensor, offset=src_row.offset,
                   ap=[[0, parts]] + [list(d) for d in src_row.ap[1:]])


def _rep3(t):
    """Repeat a [128, L] tile 3x along free dim via a stride-0 middle dim."""
    ap = t[:].ap
    return bass.AP(tensor=t[:].tensor, offset=t[:].offset,
                   ap=[list(ap[0]), [0, 3], list(ap[1])])


def _emit(nc, tc, psA, psB, dram, pp, rep, tens):
    (xh16, x_res, in_wT, convw, conv_bias, z_bias, xp_wT, dt_wT_f, dt_wT_b,
     dt_bias, d_skip, out_wT, w1T, b1_sh, w2T, b2_row, out) = tens
    L3 = 3 * L

    # ---------------- persistent small tensors ----------------
    convw_sb = pp.tile([128, 24], F32); nc.sync.dma_start(convw_sb[:], convw[:])
    conv_b_sb = pp.tile([128, 6], F32); nc.sync.dma_start(conv_b_sb[:], conv_bias[:])
    z_b_sb = pp.tile([128, 6], F32); nc.sync.dma_start(z_b_sb[:], z_bias[:])
    dt_b_sb = pp.tile([128, 6], F32); nc.sync.dma_start(dt_b_sb[:], dt_bias[:])
    d_skip_sb = pp.tile([128, 6], F32); nc.sync.dma_start(d_skip_sb[:], d_skip[:])
    b1_sb = pp.tile([128, 24], F32); nc.sync.dma_start(b1_sb[:], b1_sh[:])
    b2_32 = pp.tile([1, D_MODEL], F32); nc.sync.dma_start(b2_32[:], b2_row[:])
    b2_16 = pp.tile([1, D_MODEL], BF)
    nc.vector.tensor_copy(b2_16[:], b2_32[:])
    eps_sb = pp.tile([128, 1], F32); nc.vector.memset(eps_sb[:], 1e-5)
    ones_row = pp.tile([1, 128], BF); nc.vector.memset(ones_row[:], 1.0)
    ones_col = pp.tile([128, 1], BF); nc.vector.memset(ones_col[:], 1.0)

    xp_w_sb = pp.tile([128, 480], BF)
    nc.sync.dma_start(xp_w_sb[:], xp_wT[:])
    dtw_sb = []
    for t in (dt_wT_f, dt_wT_b):
        w = pp.tile([DT_RANK, DQ], BF)
        nc.sync.dma_start(w[:], t[:])
        dtw_sb.append(w)

    def replicate_ps(ps_tile, row_bf, width=L):
        for o in range(0, width, 512):
            w = min(512, width - o)
            nc.tensor.matmul(ps_tile[:, o:o + w], ones_row[:],
                             row_bf[0:1, o:o + w], start=True, stop=True)

    # persistent activations (fat per-direction tiles)
    z_fat = [pp.tile([128, L3], BF, tag=f"z{d}", name=f"z{d}") for d in range(2)]
    xcs_fat = [pp.tile([128, L3], BF, tag=f"xcs{d}", name=f"xcs{d}") for d in range(2)]
    delta_fat = [pp.tile([128, L3], BF, tag=f"dl{d}", name=f"dl{d}") for d in range(2)]
    dx_fat = [pp.tile([128, L3], BF, tag=f"dx{d}", name=f"dx{d}") for d in range(2)]
    acc_fat = [pp.tile([128, L3], BF, tag=f"acc{d}", name=f"acc{d}") for d in range(2)]
    yg_fat = dx_fat   # gate output reuses the dx tiles (dx dead by then)

    # collective DRAM buffers (single merged AllReduce measured faster than
    # one per direction)
    cc_in = dram.tile([160, L], AR_DTYPE, tag="cci", name="cci")
    cc_out = dram.tile([160, L], AR_DTYPE, tag="cco", name="cco")
    rs_in = dram.tile([L, D_MODEL], RS_DTYPE, tag="rsi", name="rsi")
    rs_out = dram.tile([TOK, D_MODEL], RS_DTYPE, tag="rso", name="rso")

    # ========== LN1 stats + xn + in_proj + conv + xp + delta ==========
    with tc.tile_pool(name=f"ln1_{rep}", bufs=1) as lp, \
         tc.tile_pool(name=f"work_{rep}", bufs=2) as wk:
        xh = [lp.tile([128, L], BF, tag=f"xh{k}", name=f"xh{k}")
              for k in range(6)]
        xnr = [lp.tile([128, L], BF, tag=f"xnr{k}", name=f"xnr{k}")
               for k in range(6)]
        psum_s = psA.tile([1, L], F32, tag="mm", name="lnred_s")
        psum_q = psA.tile([1, L], F32, tag="mm", name="lnred_q")
        for k in range(6):
            nc.sync.dma_start(xh[k][:], xh16[k * 128:(k + 1) * 128, :])
            sqb = wk.tile([128, L], BF, tag="sqb", name="sqb")
            nc.scalar.activation(sqb[:], xh[k][:], ACTF.Square)
            for nh in range(NH):
                nc.tensor.matmul(psum_s[:, nh * 512:(nh + 1) * 512],
                                 ones_col[:], xh[k][:, nh * 512:(nh + 1) * 512],
                                 start=(k == 0), stop=(k == 5))
                nc.tensor.matmul(psum_q[:, nh * 512:(nh + 1) * 512],
                                 ones_col[:], sqb[:, nh * 512:(nh + 1) * 512],
                                 start=(k == 0), stop=(k == 5))
        mean = wk.tile([1, L], F32, tag="vt", name="mean", bufs=3)
        nc.scalar.activation(mean[:], psum_s[:], ACTF.Copy, scale=1.0 / D_MODEL)
        mean16 = lp.tile([1, L], BF)
        nc.vector.tensor_copy(mean16[:], mean[:])
        e2 = wk.tile([1, L], F32, tag="vt", name="e2", bufs=3)
        nc.scalar.activation(e2[:], psum_q[:], ACTF.Copy, scale=1.0 / D_MODEL)
        var = wk.tile([1, L], F32, tag="vt", name="var", bufs=3)
        nc.vector.tensor_mul(var[:], mean[:], mean[:])
        nc.vector.tensor_sub(var[:], e2[:], var[:])
        sd = wk.tile([1, L], F32, tag="vt", name="sd", bufs=3)
        nc.scalar.activation(sd[:], var[:], ACTF.Sqrt, bias=eps_sb[0:1, :])
        rstd = wk.tile([1, L], F32, tag="vt", name="rstd", bufs=3)
        nc.vector.reciprocal(rstd[:], sd[:])
        rstd16 = lp.tile([1, L], BF)
        nc.vector.tensor_copy(rstd16[:], rstd[:])
        mrep_ps = psA.tile([128, L], F32, tag="mm", name="mrep_ps")
        replicate_ps(mrep_ps, mean16)
        mrep = lp.tile([128, L], BF)
        nc.scalar.activation(mrep[:], mrep_ps[:], ACTF.Copy)
        rrep_ps = psA.tile([128, L], F32, tag="mm", name="rrep_ps")
        replicate_ps(rrep_ps, rstd16)
        rrep = lp.tile([128, L], BF)
        nc.scalar.activation(rrep[:], rrep_ps[:], ACTF.Copy)
        xn = xh  # normalize in place
        for k in range(6):
            nc.vector.tensor_sub(xh[k][:], xh[k][:], mrep[:])
            nc.vector.tensor_mul(xh[k][:], xh[k][:], rrep[:])
            nc.vector.tensor_copy(xnr[k][:], xn[k][:, ::-1])

        # ---- in_proj ----
        xc_pad = [wk.tile([128, L + 4], BF, tag=f"xcp{i}", name=f"xcp{i}",
                          bufs=1)
                  for i in range(6)]
        for i in range(6):
            nc.vector.memset(xc_pad[i][:, 0:3], 0.0)
            nc.vector.memset(xc_pad[i][:, L + 3:L + 4], 0.0)
        # m order: 0-2 xc_f, 3-5 xc_b first; z tiles run during the AR wait
        with tc.tile_pool(name=f"iw_{rep}", bufs=1) as iwp:
            iwk = [iwp.tile([128, 4 * DQ], BF, tag=f"iwk{k}", name=f"iwk{k}")
                   for k in range(6)]
            for k in range(6):
                nc.sync.dma_start(iwk[k][:], in_wT[k * 128:(k + 1) * 128, :])

            def in_proj_tile(m):
                pm = psA.tile([128, L], F32, tag="mm", name="mm")
                grp, mt = divmod(m, 3)   # 0:xc_f 1:xc_b 2:z_f 3:z_b
                d = grp % 2
                rhs = xnr if d == 1 else xn
                for k in range(6):
                    for nh in range(NH):
                        nc.tensor.matmul(pm[:, nh * 512:(nh + 1) * 512],
                                         iwk[k][:, m * 128:(m + 1) * 128],
                                         rhs[k][:, nh * 512:(nh + 1) * 512],
                                         start=(k == 0), stop=(k == 5))
                i = d * 3 + mt
                if grp < 2:      # xc
                    nc.scalar.activation(xc_pad[i][:, 3:3 + L], pm[:], ACTF.Copy)
                else:            # z: apply silu right here (bias = ln_b fold)
                    nc.scalar.activation(z_fat[d][:, mt * L:(mt + 1) * L],
                                         pm[:], ACTF.Silu,
                                         bias=z_b_sb[:, i:i + 1])

            for m in range(12):
                in_proj_tile(m)

            # ---- conv + silu; xp partials; one AllReduce for both dirs ----
            for d in range(2):
                for mt in range(3):
                    i = d * 3 + mt
                    tmp = wk.tile([128, L], F32, tag="cvt", name="cvt")
                    for j in range(4):
                        nc.vector.scalar_tensor_tensor(
                            tmp[:], xc_pad[i][:, j:j + L],
                            convw_sb[:, i * 4 + j:i * 4 + j + 1], tmp[:],
                            AL.mult, AL.bypass if j == 0 else AL.add)
                    nc.scalar.activation(xcs_fat[d][:, mt * L:(mt + 1) * L],
                                         tmp[:], ACTF.Silu,
                                         bias=conv_b_sb[:, i:i + 1])
                pxp = psB.tile([80, L], F32, tag="xp", name="xp")
                for kt in range(3):
                    for nh in range(NH):
                        nc.tensor.matmul(pxp[:, nh * 512:(nh + 1) * 512],
                                         xp_w_sb[:, d * 240 + kt * 80:
                                                 d * 240 + (kt + 1) * 80],
                                         xcs_fat[d][:, kt * L + nh * 512:
                                                    kt * L + (nh + 1) * 512],
                                         start=(kt == 0), stop=(kt == 2))
                sxp = wk.tile([80, L], AR_DTYPE, tag="sxp", name=f"sxp{d}")
                nc.scalar.activation(sxp[:], pxp[:], ACTF.Copy)
                nc.sync.dma_start(cc_in[d * 80:(d + 1) * 80, :], sxp[:])
            if COLL_MODE == 'nocoll':
                nc.sync.dma_start(cc_out[:], cc_in[:])
            else:
                nc.gpsimd.collective_compute("AllReduce", AL.add,
                                             replica_groups=GROUPS,
                                             ins=[cc_in.opt()],
                                             outs=[cc_out.opt()])

        # ---- post-AR: dt matmul + softplus + dx ----
        for d in range(2):
            dt16 = wk.tile([DT_RANK, L], BF, tag="dt16", name=f"dt16{d}")
            nc.sync.dma_start(dt16[:], cc_out[d * 80:d * 80 + DT_RANK, :])
            for mt in range(3):
                i = d * 3 + mt
                pdl = psA.tile([128, L], F32, tag="mm", name="mm")
                for nh in range(NH):
                    nc.tensor.matmul(pdl[:, nh * 512:(nh + 1) * 512],
                                     dtw_sb[d][:, mt * 128:(mt + 1) * 128],
                                     dt16[:, nh * 512:(nh + 1) * 512],
                                     start=True, stop=True)
                esp = wk.tile([128, L], F32, tag="esp", name="esp")
                nc.scalar.activation(esp[:], pdl[:], ACTF.Exp,
                                     bias=dt_b_sb[:, i:i + 1])
                nc.scalar.activation(delta_fat[d][:, mt * L:(mt + 1) * L],
                                     esp[:], ACTF.Ln, bias=1.0)
            nc.vector.tensor_mul(dx_fat[d][:], delta_fat[d][:], xcs_fat[d][:])

    # ====== late weights (DMAs overlap the scan phase, Act queue) ======
    with tc.tile_pool(name=f"wf_{rep}", bufs=1) as wf:
        outw_sb = [wf.tile([128, D_MODEL], BF, tag=f"outw{k}", name=f"outw{k}")
                   for k in range(6)]
        for k in range(6):
            nc.scalar.dma_start(outw_sb[k][:], out_wT[k * 128:(k + 1) * 128, :])
        w1_sb = [wf.tile([128, 4 * D_MODEL], BF, tag=f"w1s{k}", name=f"w1s{k}")
                 for k in range(6)]
        for k in range(6):
            nc.scalar.dma_start(w1_sb[k][:], w1T[k * 128:(k + 1) * 128, :])
        w2_sb = [wf.tile([128, 4 * D_MODEL], BF, tag=f"w2s{j}", name=f"w2s{j}")
                 for j in range(6)]
        for j in range(6):
            for kk in range(4):
                nc.scalar.dma_start(
                    w2_sb[j][:, kk * D_MODEL:(kk + 1) * D_MODEL],
                    w2T[(4 * j + kk) * 128:(4 * j + kk + 1) * 128, :])

        # ============== selective scan (all forward) ==============
        with tc.tile_pool(name=f"scan_{rep}", bufs=2) as sp, \
             tc.tile_pool(name=f"rep_{rep}", bufs=3) as rp:
            for d in range(2):
                # software-pipeline the acc += ch adds two states behind the
                # scans so DVE never waits on Pool's C-multiply
                pend = []
                for s in range(D_STATE):
                    brep = rp.tile([128, L], BF, tag="brep", name="brep")
                    nc.sync.dma_start(
                        brep[:],
                        _bcast(cc_out[d * 80 + DT_RANK + s:
                                      d * 80 + DT_RANK + s + 1, :]))
                    crep = rp.tile([128, L], BF, tag="crep", name="crep")
                    nc.sync.dma_start(
                        crep[:],
                        _bcast(cc_out[d * 80 + DT_RANK + D_STATE + s:
                                      d * 80 + DT_RANK + D_STATE + s + 1, :]))
                    dA = sp.tile([128, 3 * L], BF, tag="dA", name="dA")
                    nc.scalar.activation(dA[:], delta_fat[d][:], ACTF.Exp,
                                         scale=-(s + 1.0))
                    dBu = sp.tile([128, 3 * L], BF, tag="dBu", name="dBu")
                    nc.vector.tensor_tensor(dBu[:], dx_fat[d][:], _rep3(brep),
                                            AL.mult)
                    h = sp.tile([128, 3 * L], BF, tag="h", name="h")
                    for mt in range(3):
                        nc.vector.tensor_tensor_scan(
                            h[:, mt * L:(mt + 1) * L],
                            dA[:, mt * L:(mt + 1) * L],
                            dBu[:, mt * L:(mt + 1) * L],
                            0.0, AL.mult, AL.add)
                    if s == 0:
                        nc.gpsimd.tensor_tensor(acc_fat[d][:], h[:],
                                                _rep3(crep), AL.mult)
                    else:
                        ch = sp.tile([128, 3 * L], BF, tag="ch", name="ch")
                        nc.gpsimd.tensor_tensor(ch[:], h[:], _rep3(crep),
                                                AL.mult)
                        pend.append(ch)
                    if pend:
                        nc.vector.tensor_add(acc_fat[d][:], acc_fat[d][:],
                                             pend.pop(0)[:])

        # ---- gating + out_proj + ReduceScatter ----
        with tc.tile_pool(name=f"gate_{rep}", bufs=2) as gp:
            for d in range(2):
                for mt in range(3):
                    i = d * 3 + mt
                    sl_ = slice(mt * L, (mt + 1) * L)
                    tmp = gp.tile([128, L], BF, tag="gt", name="gt")
                    nc.vector.scalar_tensor_tensor(
                        tmp[:], xcs_fat[d][:, sl_], d_skip_sb[:, i:i + 1],
                        acc_fat[d][:, sl_], AL.mult, AL.add)
                    if d == 0:
                        nc.vector.tensor_mul(yg_fat[d][:, sl_], tmp[:],
                                             z_fat[d][:, sl_])
                    else:
                        # backward dir: un-reverse while writing
                        nc.vector.tensor_mul(
                            yg_fat[d][:, (mt + 1) * L - 1:mt * L - 1 if mt else None:-1],
                            tmp[:], z_fat[d][:, sl_])

        with tc.tile_pool(name=f"opj_{rep}", bufs=2) as opj:
            for tt in range(8):
                po = psA.tile([128, D_MODEL], F32, tag="mm", name="po")
                for ki in range(6):
                    d, mt = divmod(ki, 3)
                    lhs = yg_fat[d][:, mt * L + tt * 128:mt * L + (tt + 1) * 128]
                    for o, w in ((0, 512), (512, 256)):
                        nc.tensor.matmul(po[:, o:o + w], lhs,
                                         outw_sb[ki][:, o:o + w],
                                         start=(ki == 0), stop=(ki == 5))
                so = opj.tile([128, D_MODEL], RS_DTYPE, tag="so", name="so")
                nc.scalar.activation(so[:], po[:], ACTF.Copy)
                nc.sync.dma_start(rs_in[tt * 128:(tt + 1) * 128, :], so[:])
        # ======= residual + LN2 (token-major) + FFN =======
        with tc.tile_pool(name=f"ffn_{rep}", bufs=1) as fp:
            # x_res tiles don't depend on the collective: prefetch them now
            xr = [fp.tile([128, D_MODEL], F32, tag=f"xr{t}", name=f"xr{t}")
                  for t in range(2)]
            for t in range(2):
                nc.sync.dma_start(xr[t][:], x_res[t * 128:(t + 1) * 128, :])
            if COLL_MODE == 'nocoll':
                nc.sync.dma_start(rs_out[:], rs_in[0:TOK, :])
            else:
                nc.gpsimd.collective_compute("ReduceScatter", AL.add,
                                             replica_groups=GROUPS,
                                             ins=[rs_in.opt()],
                                             outs=[rs_out.opt()])
            x2 = [fp.tile([128, D_MODEL], F32, tag=f"x2{t}", name=f"x2{t}")
                  for t in range(2)]
            for t in range(2):
                rsy = fp.tile([128, D_MODEL], RS_DTYPE, tag="rsy", name="rsy")
                nc.sync.dma_start(rsy[:], rs_out[t * 128:(t + 1) * 128, :])
                nc.vector.tensor_add(x2[t][:], rsy[:], xr[t][:])
            xn2_bf = [fp.tile([128, D_MODEL], BF, tag=f"xn2{t}", name=f"xn2{t}")
                      for t in range(2)]
            for t in range(2):
                stats = fp.tile([128, 3, 6], F32, tag="bst", name="bst")
                for c in range(3):
                    nc.vector.bn_stats(stats[:, c, :],
                                       x2[t][:, c * 256:(c + 1) * 256])
                mv = fp.tile([128, 2], F32, tag="mv", name="mv")
                nc.vector.bn_aggr(mv[:], stats[:])
                sd2 = fp.tile([128, 1], F32, tag="sd2", name="sd2")
                nc.scalar.activation(sd2[:], mv[:, 1:2], ACTF.Sqrt,
                                     bias=eps_sb[:, 0:1])
                rstd2 = fp.tile([128, 1], F32, tag="rstd2", name="rstd2")
                nc.vector.reciprocal(rstd2[:], sd2[:])
                t1 = fp.tile([128, D_MODEL], F32, tag="ft1", name="ft1")
                nc.vector.tensor_scalar_sub(t1[:], x2[t][:], mv[:, 0:1])
                nc.vector.tensor_scalar_mul(xn2_bf[t][:], t1[:], rstd2[:])
            # transpose xn2 to feature-major via xbar DMA
            xn2_fm = [fp.tile([128, TOK], BF, tag=f"x2f{j}", name=f"x2f{j}")
                      for j in range(6)]
            for j in range(6):
                for t in range(2):
                    nc.sync.dma_start_transpose(
                        xn2_fm[j][:, t * 128:(t + 1) * 128],
                        xn2_bf[t][:, j * 128:(j + 1) * 128])
            # mm1 + gelu -> h_fm [3072, 256] bf16
            h_fm = [fp.tile([128, TOK], BF, tag=f"hf{m}", name=f"hf{m}")
                    for m in range(24)]
            for m in range(24):
                pf = psA.tile([128, TOK], F32, tag="mm", name="pf")
                for k in range(6):
                    nc.tensor.matmul(pf[:], w1_sb[k][:, m * 128:(m + 1) * 128],
                                     xn2_fm[k][:], start=(k == 0), stop=(k == 5))
                nc.scalar.activation(h_fm[m][:], pf[:], ACTF.Gelu,
                                     bias=b1_sb[:, m:m + 1])
            # mm2 (token-major out) with b2 as an augmented K row
            for t in range(2):
                po2 = psA.tile([128, D_MODEL], F32, tag="mm", name=f"po2{t}")
                for k in range(24):
                    j, kk = divmod(k, 4)
                    for o, w in ((0, 512), (512, 256)):
                        nc.tensor.matmul(
                            po2[:, o:o + w],
                            h_fm[k][:, t * 128:(t + 1) * 128],
                            w2_sb[j][:, kk * D_MODEL + o:kk * D_MODEL + o + w],
                            start=(k == 0), stop=False)
                for o, w in ((0, 512), (512, 256)):
                    nc.tensor.matmul(po2[:, o:o + w], ones_row[:],
                                     b2_16[0:1, o:o + w],
                                     start=False, stop=True)
                t4 = fp.tile([128, D_MODEL], F32, tag="t4", name="t4")
                nc.vector.tensor_add(t4[:], po2[:], x2[t][:])
                nc.sync.dma_start(out[t * 128:(t + 1) * 128, :], t4[:])


def build():
    nc = bacc.Bacc("TRN2", target_bir_lowering=False, debug=False,
                   num_devices=NCORES)

    def din(name, shape, dt=F32):
        return nc.dram_tensor(name, shape, dt, kind="ExternalInput")

    xh16 = din("xh16", [D_MODEL, L], BF)            # x[b].T  (bf16)
    x_res = din("x_res", [TOK, D_MODEL])            # token slice of x[b]
    in_wT = din("in_wT", [D_MODEL, 4 * DQ], BF)     # m: xc_f xc_b z_f z_b
    convw = din("convw", [128, 24])                 # (tile, tap)
    conv_bias = din("conv_bias", [128, 6])          # silu bias after conv
    z_bias = din("z_bias", [128, 6])                # silu bias for z
    xp_wT = din("xp_wT", [128, 480], BF)            # 2 dirs x 3 k-tiles
    dt_wT_f = din("dt_wT_f", [DT_RANK, DQ], BF)
    dt_wT_b = din("dt_wT_b", [DT_RANK, DQ], BF)
    dt_bias = din("dt_bias", [128, 6])
    d_skip = din("d_skip", [128, 6])
    out_wT = din("out_wT", [2 * DQ, D_MODEL], BF)   # rows: f then b, x0.5
    w1T = din("w1T", [D_MODEL, 4 * D_MODEL], BF)    # ln2-g folded
    b1_sh = din("b1_sh", [128, 24])                 # b1 + w1 @ ln2-b
    w2T = din("w2T", [4 * D_MODEL, D_MODEL], BF)
    b2_row = din("b2_row", [1, D_MODEL])
    out = nc.dram_tensor("out", [TOK, D_MODEL], F32, kind="ExternalOutput")
    tens = (xh16, x_res, in_wT, convw, conv_bias, z_bias, xp_wT, dt_wT_f,
            dt_wT_b, dt_bias, d_skip, out_wT, w1T, b1_sh, w2T, b2_row, out)

    with tile.TileContext(nc) as tc:
        with tc.tile_pool(name="psA", bufs=3, space="PSUM") as psA, \
             tc.tile_pool(name="psB", bufs=1, space="PSUM") as psB, \
             tc.tile_pool(name="dram", bufs=1, space="DRAM") as dram:
            for rep in range(N_REPS):
                with tc.tile_pool(name=f"persist_{rep}", bufs=1) as pp:
                    _emit(nc, tc, psA, psB, dram, pp, rep, tens)

    nc.compile()
    return nc


def _prep(inputs):
    f32 = np.float32
    x = np.asarray(inputs['x'], f32)
    ln_g = np.asarray(inputs['ln_g'], f32)
    ln_b = np.asarray(inputs['ln_b'], f32)
    g2 = np.asarray(inputs['ffn_ln_g'], f32)
    b2ln = np.asarray(inputs['ffn_ln_b'], f32)
    w1 = np.asarray(inputs['w1'], f32)
    b1 = np.asarray(inputs['b1'], f32)
    w2 = np.asarray(inputs['w2'], f32)
    b2 = np.asarray(inputs['b2'], f32)

    maps = []
    for core in range(NCORES):
        b, q = divmod(core, NQ)
        sl = slice(q * DQ, (q + 1) * DQ)

        def pp(v):  # (768,) -> (128, 6) per-partition columns
            return np.ascontiguousarray(v.reshape(6, 128).T.astype(f32))

        m = {}
        m['xh16'] = np.ascontiguousarray(x[b].T).astype(BF_NP)
        m['x_res'] = np.ascontiguousarray(x[b, q * TOK:(q + 1) * TOK])

        # in_proj weights with ln_g folded; column order xc_f xc_b z_f z_b
        iw_f = np.asarray(inputs['in_w_f'], f32)
        iw_b = np.asarray(inputs['in_w_b'], f32)
        zsl = slice(D_INNER + q * DQ, D_INNER + (q + 1) * DQ)
        xc_f_w = iw_f[sl] * ln_g[None, :]
        z_f_w = iw_f[zsl] * ln_g[None, :]
        xc_b_w = iw_b[sl] * ln_g[None, :]
        z_b_w = iw_b[zsl] * ln_g[None, :]
        m['in_wT'] = np.concatenate([xc_f_w, xc_b_w, z_f_w, z_b_w]).T.astype(BF_NP)
        # ln_b contribution (constant per channel)
        c0_xc_f = iw_f[sl] @ ln_b
        c0_z_f = iw_f[zsl] @ ln_b
        c0_xc_b = iw_b[sl] @ ln_b
        c0_z_b = iw_b[zsl] @ ln_b

        # conv: natural taps both dirs (bwd input is time-reversed)
        wf_ = np.asarray(inputs['conv_w_f'], f32)[sl, 0, :]
        wb_ = np.asarray(inputs['conv_w_b'], f32)[sl, 0, :]
        W = np.concatenate([wf_, wb_])
        cw = np.zeros((128, 24), f32)
        for i in range(6):
            cw[:, i * 4:(i + 1) * 4] = W[i * 128:(i + 1) * 128]
        m['convw'] = cw
        cb_f = np.asarray(inputs['conv_b_f'], f32)[sl] + c0_xc_f * wf_.sum(1)
        cb_b = np.asarray(inputs['conv_b_b'], f32)[sl] + c0_xc_b * wb_.sum(1)
        m['conv_bias'] = pp(np.concatenate([cb_f, cb_b]))
        m['z_bias'] = pp(np.concatenate([c0_z_f, c0_z_b]))

        def pack_xp(w):  # (80, 1536) -> [128, 240] (3 k-tiles of [128,80])
            wt = w[:, sl].T.astype(BF_NP)        # [384, 80]
            out_ = np.zeros((128, 240), BF_NP)
            for kt in range(3):
                out_[:, kt * 80:(kt + 1) * 80] = wt[kt * 128:(kt + 1) * 128]
            return out_
        m['xp_wT'] = np.concatenate(
            [pack_xp(np.asarray(inputs['xp_w_f'], f32)),
             pack_xp(np.asarray(inputs['xp_w_b'], f32))], axis=1)
        m['dt_wT_f'] = np.asarray(inputs['dt_w_f'], f32)[sl].T.astype(BF_NP)
        m['dt_wT_b'] = np.asarray(inputs['dt_w_b'], f32)[sl].T.astype(BF_NP)
        m['dt_bias'] = pp(np.concatenate([np.asarray(inputs['dt_b_f'], f32)[sl],
                                          np.asarray(inputs['dt_b_b'], f32)[sl]]))
        m['d_skip'] = pp(np.concatenate([np.asarray(inputs['D_f'], f32)[sl],
                                         np.asarray(inputs['D_b'], f32)[sl]]))
        ow = np.concatenate([np.asarray(inputs['out_w_f'], f32)[:, sl].T,
                             np.asarray(inputs['out_w_b'], f32)[:, sl].T]) * 0.5
        m['out_wT'] = ow.astype(BF_NP)

        # FFN with ln2 folds
        m['w1T'] = (w1 * g2[None, :]).T.astype(BF_NP)
        b1p = b1 + w1 @ b2ln
        m['b1_sh'] = np.ascontiguousarray(b1p.reshape(24, 128).T)
        m['w2T'] = w2.T.astype(BF_NP)
        m['b2_row'] = b2[None, :]
        maps.append({k: np.ascontiguousarray(v) for k, v in m.items()})
    return maps


def kernel(**inputs):
    if 'nc' not in _CACHE:
        _CACHE['nc'] = build()
    nc = _CACHE['nc']
    maps = _prep(inputs)
    res = run_bass_kernel_spmd(nc, maps, core_ids=list(range(NCORES)), trace=False)
    out = np.empty((B_SZ, L, D_MODEL), np.float32)
    for core in range(NCORES):
        b, q = divmod(core, NQ)
        out[b, q * TOK:(q + 1) * TOK] = res.results[core]['out']
    return out
